# revision 45
# baseline (speedup 1.0000x reference)
"""Trainium2 Bass kernel for AltAttention (B=2, S=2048, D=1024, 16 heads).

Distribution over 8 NeuronCores: data-parallel over batch (2) x
tensor-parallel over heads (4 heads/core).

Per-core pipeline (cost-model-aware design):
  - QKV projection chains (PSUM accumulation over D). The hp0 (heads
    0-1) q/k chains read an fp8 copy of x^T that is DMA'd first (half
    the bytes of fp16), so the first scores tile is ready ~8.5us in;
    v chains and hp1 chains read the fp16 x^T for accuracy.
  - q/k evacuated to fp8e4m3 in a [64, 2, S] layout (hd split over the
    two DoubleRow k-subtiles; subtile 1 zero-padded) with bias and
    sqrt(1/32)-scale folded in, so the scores matmul runs in fp8
    DoubleRow mode at 0.5 cycles/row. The h-even strip evacuation is
    emitted before the h-odd one so the first unit's scores gate on a
    single evac chain.
  - Scores land as [128 k, 1024 q] PSUM tiles; the ScalarE exp stream
    (128 x [128,1024] activations, ~133 us) is the critical engine; a
    build-time virtual-clock scheduler (with a 2-stage HWDGE+transfer
    DMA queue model) paces every other engine's emission so the exp
    stream never starves.
  - PV is *flipped*: pt slices [k,q] are the stationary operand, v
    strips [k,64] the moving one, so each accumulation step streams
    only 64+1 rows. Accumulators for 8 q-tiles share one PSUM bank via
    memset + start=False accumulation; a parallel 1-column matmul
    accumulates the softmax denominators.
  - Normalize (DVE broadcast multiply by 1/denom), PE-transpose to
    [hd,q], output projection, fp16 partial-y DMA; host sums 4
    partials per batch and adds b_proj. The last unit's tail runs as
    4 per-qt-pair waves with evacuation work round-robined between
    ScalarE (idle after the last exp) and DVE.
"""
import numpy as np

import concourse.bacc as bacc
import concourse.mybir as mybir
from concourse.tile import TileContext
from concourse.bass_utils import run_bass_kernel_spmd

B = 2
S = 2048
D = 1024
H = 16
HD = 64
SCALE = D ** (-0.5)
RS = SCALE ** 0.5          # sqrt-scale folded into both q and k
N_CORES = 8
TP = 4                     # heads per core
F32 = mybir.dt.float32
F16 = mybir.dt.float16
F8 = mybir.dt.float8e4
EXP = mybir.ActivationFunctionType.Exp
DR = mybir.MatmulPerfMode.DoubleRow
MUL = mybir.AluOpType.mult
ADD = mybir.AluOpType.add

KO = D // 128              # 8 contraction tiles over D
KT = S // 128              # 16 key tiles
QC = 2                     # q chunks of 1024
QW = S // QC               # 1024
NQT = QW // 128            # 8 q-subtiles per chunk

# virtual-clock cost constants (ns, slightly pessimistic for PE)
C_SCORES = 222             # 2 DoubleRow mms [512] @ 106.7 + decode
C_PV = 250                 # 8x(64-mm + 1-mm) + decode
C_CHAIN4 = 880             # 4 x 512-mm
C_V4 = 440                 # 4 x 256-mm
C_V5 = 550                 # 4 x 256-mm + bias mm
C_PROJ = 440               # 2 x 512-mm
C_TRANS = 56               # 128-row fp16 transpose
C_EXP = 1041               # [128,1024] exp from PSUM
C_EVAC = 660               # DVE 512-free PSUM-read op
C_VEVAC = 394              # DVE 256-free PSUM-read op
C_RECIP = 135
C_NORM = 660
C_MEMA = 660
C_MEMD = 135
C_TCOPY = 260              # DVE [128,128] psum->sbuf copy
SEM = 250                  # cross-engine semaphore latency
WARG = 250                 # scheduler reserve margin


DEBUG_EXP = []
DEBUG_PV = []
DEBUG_INFO = {}


def _build():
    DEBUG_EXP.clear()
    DEBUG_PV.clear()
    nc = bacc.Bacc("TRN2", target_bir_lowering=False, debug=False,
                   num_devices=N_CORES)

    xT = nc.dram_tensor("xT", [D, S], F16, kind="ExternalInput")
    xT8 = nc.dram_tensor("xT8", [D, S], F8, kind="ExternalInput")
    # per-core weight slices, host-prearranged:
    #   wqk: [D, 512] cols = [q-hp0 | k-hp0 | q-hp1 | k-hp1] (128 each)
    #   wv:  [D, 256] cols = v h0..h3
    wqk = nc.dram_tensor("wqk", [D, 512], F16, kind="ExternalInput")
    wv = nc.dram_tensor("wv", [D, 256], F16, kind="ExternalInput")
    wp = nc.dram_tensor("wp", [TP * HD, D], F16, kind="ExternalInput")
    # bqk[:, c]: per-partition bias (pre-scaled by RS): cols q-hp0,q-hp1,k-hp0,k-hp1
    bqk = nc.dram_tensor("bqk", [128, 4], F32, kind="ExternalInput")
    bv = nc.dram_tensor("bv", [1, 256], F16, kind="ExternalInput")
    onec_in = nc.dram_tensor("onec_in", [128, 1], F16, kind="ExternalInput")
    eye_in = nc.dram_tensor("eye_in", [128, 128], F16, kind="ExternalInput")
    y = nc.dram_tensor("y", [S, D], F16, kind="ExternalOutput")
    # kf0 partial of the qc1 rows, computed mid-stream; host adds to y rows
    y2 = nc.dram_tensor("y2", [QW, D], F16, kind="ExternalOutput")

    with TileContext(nc) as tc, \
         nc.allow_low_precision(reason="fp16/fp8 PE operands; fp16 partial y"):
        with tc.tile_pool(name="pconst", bufs=1) as pc, \
             tc.tile_pool(name="pmain", bufs=1) as pm, \
             tc.tile_pool(name="pp", bufs=1, space="PSUM") as pp:
            # ---- resident constants / weights ----
            wqk_sb = pc.tile([128, KO, 512], F16, name="wqk_sb")
            wv_sb = pc.tile([128, KO, 256], F16, name="wv_sb")
            wp_sb = pc.tile([128, 2 * D], F16, name="wp_sb")
            bqk_sb = pc.tile([128, 4], F32, name="bqk_sb")
            bv_sb = pc.tile([1, 256], F16, name="bv_sb")
            onec_sb = pc.tile([128, 1], F16, name="onec_sb")
            eye_sb = pc.tile([128, 128], F16, name="eye_sb")
            junk_sb = pc.tile([128, 512], F16, name="junk_sb")
            junk2_sb = pc.tile([128, 128], F16, name="junk2_sb")

            # ---- persistent activations ----
            xT_sb = pm.tile([128, KO, S], F16, name="xT_sb")
            xT8_sb = pm.tile([128, KO, S], F8, name="xT8_sb")
            qT = [pm.tile([64, 2, S], F8, name=f"qT{h}") for h in range(TP)]
            kTt = [pm.tile([64, 2, S], F8, name=f"kT{h}") for h in range(TP)]
            v_view = pm.tile([128, KT, TP, HD], F16, name="v_aug")
            attn_sb = [pm.tile([128, NQT, 128], F16, name=f"attn{hp}")
                       for hp in range(2)]
            attnT = [pm.tile([128, S], F16, name=f"attnT{hp}")
                     for hp in range(2)]
            rec_sb = pm.tile([128, 2, NQT], F32, name="rec_sb")

            with tc.tile_pool(name="pwork", bufs=1) as pw:
                # ---- virtual clocks ----
                vt = {"pe": 0.0, "act": 0.0, "dve": 0.0}
                dmaq = {"hw": 700.0, "t": 0.0}

                def pe_op(cost, gate=0.0):
                    vt["pe"] = max(vt["pe"], gate) + cost
                    return vt["pe"]

                def dve_op(cost, gate=0.0):
                    vt["dve"] = max(vt["dve"], gate) + cost
                    return vt["dve"]

                def act_op(cost, gate=0.0):
                    vt["act"] = max(vt["act"], gate) + cost
                    return vt["act"]

                def dma_op(pp_bytes, elem_bytes=512):
                    # 2-stage queue: serial HWDGE descriptor gen (625ns),
                    # serial DMA transfer at ~0.3555 ns/B-per-partition
                    # (2x for <512B contiguous runs), then +941ns before
                    # the completion semaphore is visible to consumers.
                    dmaq["hw"] += 625.0
                    start = max(dmaq["t"], dmaq["hw"] + 650.0)
                    mult = 2.0 if elem_bytes < 512 else 1.0
                    dmaq["t"] = start + max(pp_bytes * 0.3555 * mult, 56.0)
                    return dmaq["t"] + 941.0

                acc = pp.tile([128, 512], F32, tag="acc", bufs=1, name="acc")
                den = pp.tile([128, NQT, 2], F32, tag="den", bufs=1,
                              name="den")

                # =========== warmup: PE p-state + ACT exp table =============
                nc.gpsimd.memset(junk_sb[:, :], 0.125)
                wups = [pp.tile([128, 512], F32, tag="ch", bufs=2,
                                name="wup") for _ in range(2)]
                for i in range(5):
                    nc.tensor.matmul(wups[i % 2][:, :], junk_sb[:, 0:128],
                                     junk_sb[:, :], start=True, stop=True)
                nc.scalar.activation(junk2_sb[:, :], junk_sb[:, 0:128], EXP)

                # =========== input DMAs =====================================
                # ordered so the first unit's (qc0, h0) dependencies land
                # first: w-q-hp0, bias, x8 st0, w-k-hp0, x8 st1, ...
                wqkr = wqk.rearrange("(k p) c -> p k c", p=128)
                xTr = xT.rearrange("(k p) s -> p k s", p=128)
                xT8r = xT8.rearrange("(k p) s -> p k s", p=128)
                dma_t = {}
                x8_t = {}
                # 256-col weight slices keep DRAM rows 512B-contiguous
                nc.sync.dma_start(out=wqk_sb[:, 0:4, 0:256],
                                  in_=wqkr[:, 0:4, 0:256])
                dma_t["w0a"] = dma_op(2048)
                nc.sync.dma_start(
                    out=xT8_sb[:, 0:4, 0:512], in_=xT8r[:, 0:4, 0:512])
                x8_t[(0, "a")] = dma_op(2048)
                nc.sync.dma_start(
                    out=xT8_sb[:, 4:8, 0:512], in_=xT8r[:, 4:8, 0:512])
                x8_t[(0, "b")] = x8_t[0] = dma_op(2048)
                nc.sync.dma_start(out=wqk_sb[:, 4:8, 0:256],
                                  in_=wqkr[:, 4:8, 0:256])
                dma_t["w0b"] = dma_op(2048)
                nc.sync.dma_start(out=bqk_sb[:], in_=bqk[:, :])
                dma_op(16, 16)
                for half in (0, 1):
                    nc.sync.dma_start(
                        out=xT8_sb[:, 4 * half:4 * half + 4, 512:1024],
                        in_=xT8r[:, 4 * half:4 * half + 4, 512:1024])
                    x8_t[(1, "ab"[half])] = dma_op(2048)
                x8_t[1] = x8_t[(1, "b")]
                for st in (2, 3):
                    nc.sync.dma_start(
                        out=xT8_sb[:, :, st * 512:(st + 1) * 512],
                        in_=xT8r[:, :, st * 512:(st + 1) * 512])
                    x8_t[st] = dma_op(4096)
                nc.sync.dma_start(out=bv_sb[:], in_=bv[:, :])
                dma_op(512)
                nc.sync.dma_start(out=onec_sb[:], in_=onec_in[:, :])
                dma_op(2, 2)
                nc.sync.dma_start(out=wv_sb[:, :, :],
                                  in_=wv.rearrange("(k p) c -> p k c", p=128))
                dma_t["wv"] = dma_op(4096)
                nc.sync.dma_start(
                    out=xT_sb[:, :, 0:512], in_=xTr[:, :, 0:512])
                dma_t["xc0"] = dma_op(8192)
                nc.sync.dma_start(out=wqk_sb[:, :, 256:512],
                                  in_=wqkr[:, :, 256:512])
                dma_t["w1"] = dma_op(4096)
                nc.sync.dma_start(out=eye_sb[:], in_=eye_in[:, :])
                dma_op(256, 256)
                for ch in (1, 2, 3):
                    nc.sync.dma_start(
                        out=xT_sb[:, :, ch * 512:(ch + 1) * 512],
                        in_=xTr[:, :, ch * 512:(ch + 1) * 512])
                    dma_t[f"xc{ch}"] = dma_op(8192)
                for kf in range(2):
                    nc.sync.dma_start(out=wp_sb[:, kf * D:(kf + 1) * D],
                                      in_=wp[kf * 128:(kf + 1) * 128, :])
                    dma_t[f"wp{kf}"] = dma_op(2048)

                # zero the unused DoubleRow k-subtile once (SBUF, Pool ok)
                # h0 strips first: the first unit's scores read them
                for h in range(TP):
                    nc.gpsimd.memset(qT[h][0:64, 1, :], 0.0)
                    nc.gpsimd.memset(kTt[h][0:64, 1, :], 0.0)
                # zero accumulators before first use (HW PSUM is garbage)
                nc.vector.memset(acc[:, :], 0.0)
                nc.vector.memset(den[:, :, :], 0.0)
                memset_done = {0: 900.0}

                # =========== building blocks ================================
                # wqk_sb col layout: hp*256 + (0 if q else 128)
                strip_ready = {}   # ('q'|'k', h, st) / ('v', st) -> dve time
                norm_done = {}
                attnT_ready = {}   # (hp, qc, qt) -> dve time of copy
                sched = {"drain": False, "slot": 0}

                def chain_tile():
                    return pp.tile([128, 512], F32, tag="ch", bufs=2,
                                   name="chps")

                def qk_chain_parts(hp, is_q, st, scps=False,
                                   act_evac=False):
                    """parts 0-3: 2 ko-steps each; part 3 evacuates the
                    strips (h-even: DVE cast with per-partition bias;
                    weights are RS-prescaled). act_evac (startup only):
                    the h-odd bias is added in-PSUM by a masked bias-row
                    matmul so the h-odd evac is a pure ScalarE copy.
                    scps borrows an sc-pool tile."""
                    col = hp * 256 + (0 if is_q else 128)
                    if hp == 0:
                        src = xT8_sb
                        wa, wb = dma_t["w0a"], dma_t["w0b"]
                        if st <= 1:
                            xa, xb = x8_t[(st, 'a')], x8_t[(st, 'b')]
                        else:
                            xa = xb = x8_t[st]
                        gates = [max(wa, xa) + SEM] * 2 + \
                            [max(wb, xb) + SEM] * 2
                    else:
                        src = xT_sb
                        wt = dma_t["w1"]
                        gates = [max(wt, dma_t[f"xc{st}"]) + SEM] * 4
                    gate = gates[0]
                    box = {}

                    def evac(sub):
                        h = 2 * hp + sub
                        blk = 0 if is_q else 1
                        dstl = qT if is_q else kTt
                        dst = dstl[h][0:64, 0, st * 512:(st + 1) * 512]
                        srcp = box['ps'][64 * sub:64 * sub + 64, 0:512]
                        if act_evac == "all" or (sub == 1 and act_evac):
                            nc.scalar.activation(
                                dst, srcp,
                                mybir.ActivationFunctionType.Identity,
                                bias=bqk_sb[64 * sub:64 * sub + 64,
                                            2 * blk + hp:2 * blk + hp + 1])
                            strip_ready[('q' if is_q else 'k', h, st)] = \
                                act_op(612, box['t_pe'] + SEM)
                            return
                        bias = bqk_sb[64 * sub:64 * sub + 64,
                                      2 * blk + hp:2 * blk + hp + 1]
                        nc.vector.tensor_scalar(
                            out=dst, in0=srcp, scalar1=bias, scalar2=None,
                            op0=ADD)
                        strip_ready[('q' if is_q else 'k', h, st)] = \
                            dve_op(C_EVAC, box['t_pe'] + SEM)

                    def part(p):
                        def f():
                            if p == 0:
                                if scps:
                                    box['ps'] = pp.tile([128, QW], F32,
                                                        tag="sc", bufs=2,
                                                        name="chsc")
                                else:
                                    box['ps'] = chain_tile()
                            ps = box['ps']
                            for ko in (2 * p, 2 * p + 1):
                                nc.tensor.matmul(
                                    ps[:, 0:512],
                                    wqk_sb[:, ko, col:col + 128],
                                    src[:, ko, st * 512:(st + 1) * 512],
                                    start=(ko == 0), stop=(ko == KO - 1))
                            box['t_pe'] = pe_op(C_CHAIN4 / 2, gates[p])
                            if p == 3:
                                evac(0)
                                evac(1)
                        return ("chp", f, lambda: gate)
                    return [part(p) for p in range(4)]

                def v_chain_parts(st):
                    gate = max(dma_t["wv"], dma_t[f"xc{st // 4}"]) + SEM
                    box = {}

                    def f0():
                        box['ps'] = chain_tile()
                        for ko in range(4):
                            nc.tensor.matmul(
                                box['ps'][:, 0:256],
                                xT_sb[:, ko, st * 128:(st + 1) * 128],
                                wv_sb[:, ko, :],
                                start=(ko == 0), stop=False)
                        pe_op(C_V4, gate)

                    def f1():
                        ps = box['ps']
                        for ko in range(4, 8):
                            nc.tensor.matmul(
                                ps[:, 0:256],
                                xT_sb[:, ko, st * 128:(st + 1) * 128],
                                wv_sb[:, ko, :],
                                start=False, stop=False)
                        nc.tensor.matmul(ps[:, 0:256], junk_sb[0:1, 0:128],
                                         bv_sb[0:1, :], start=False,
                                         stop=True)
                        t_pe = pe_op(C_V5)
                        nc.vector.tensor_copy(
                            v_view[:, st, :, :],
                            ps.rearrange("p (h c) -> p h c", c=HD)[:, 0:4, :])
                        strip_ready[('v', st)] = dve_op(C_VEVAC, t_pe + SEM)
                    return [("vp", f0, lambda: gate), ("vp", f1, lambda: gate)]

                def scores(h, qc, kt, sc):
                    for nn in range(2):
                        nc.tensor.matmul(
                            sc[:, nn * 512:(nn + 1) * 512],
                            kTt[h][0:64, :, kt * 128:(kt + 1) * 128],
                            qT[h][0:64, :,
                                  qc * QW + nn * 512:qc * QW + (nn + 1) * 512],
                            start=True, stop=True, perf_mode=DR)

                def pv_mms(h, kt, pt, last):
                    for qt in range(NQT):
                        st_ap = pt[:, qt * 128:(qt + 1) * 128]
                        nc.tensor.matmul(
                            acc[:, qt * HD:(qt + 1) * HD],
                            st_ap, v_view[:, kt, h, :],
                            start=False, stop=last, skip_group_check=True)
                        nc.tensor.matmul(
                            den[:, qt, h % 2:h % 2 + 1],
                            st_ap, onec_sb[:, 0:1],
                            start=False, stop=last, skip_group_check=True)

                def tail_dve(uidx, t_pv):
                    """end-of-unit: normalize, reset accumulators, queue
                    transposes. Used for all units except the last."""
                    qc, h = UNITS[uidx]
                    hp, sub = h // 2, h % 2
                    nc.vector.reciprocal(rec_sb[:, sub, :], den[:, :, sub])
                    dve_op(C_RECIP, t_pv + SEM)
                    rec_bc = rec_sb[:, sub, :].rearrange(
                        "p (q o) -> p q o", o=1).broadcast_to((128, NQT, HD))
                    acc_v = acc.rearrange("p (q c) -> p q c", c=HD)
                    nc.vector.tensor_tensor(
                        out=attn_sb[hp][:, :, sub * HD:(sub + 1) * HD],
                        in0=acc_v, in1=rec_bc, op=MUL)
                    norm_done[uidx] = dve_op(C_NORM)
                    nc.vector.memset(acc[:, :], 0.0)
                    dve_op(C_MEMA)
                    nc.vector.memset(den[:, :, sub], 0.0)
                    memset_done[uidx + 1] = dve_op(C_MEMD)
                    if h % 2 == 1:
                        for qt in range(NQT):
                            trans_queue.append((hp, qc, qt, uidx))

                def emit_transpose(hp, qc, qt, uidx):
                    tr = pp.tile([128, 128], F16, tag="ch", bufs=2, name="tr")
                    nc.tensor.transpose(tr[:, :], attn_sb[hp][:, qt, :],
                                        eye_sb[:, :])
                    t_pe = pe_op(C_TRANS, norm_done[uidx] + SEM)
                    dst = attnT[hp][:, qc * QW + qt * 128:
                                    qc * QW + (qt + 1) * 128]
                    nc.vector.tensor_copy(dst, tr[:, :])
                    attnT_ready[(hp, qc, qt)] = dve_op(C_TCOPY, t_pe + SEM)

                def proj_half(st, nn, y_sb):
                    qc, qt = st // NQT, st % NQT
                    gate = max(attnT_ready[(0, qc, qt)],
                               attnT_ready[(1, qc, qt)],
                               dma_t["wp1"]) + SEM
                    psy = pp.tile([128, 512], F32, tag="ch", bufs=2,
                                  name="psy")
                    for kf in range(2):
                        nc.tensor.matmul(
                            psy[:, :],
                            attnT[kf][:, st * 128:(st + 1) * 128],
                            wp_sb[:, kf * D + nn * 512:kf * D + nn * 512 + 512],
                            start=(kf == 0), stop=(kf == 1))
                    t_pe = pe_op(C_PROJ, gate)
                    nc.vector.tensor_copy(
                        y_sb[:, nn * 512:(nn + 1) * 512], psy[:, :])
                    dve_op(C_EVAC, t_pe + SEM)
                    if nn == 1:
                        nc.sync.dma_start(out=y[st * 128:(st + 1) * 128, :],
                                          in_=y_sb[:, :])
                        dma_op(2048)

                # =========== unit / slot bookkeeping ========================
                UNITS = [(qc, h) for qc in range(QC) for h in range(TP)]
                n_slots = len(UNITS) * KT
                sc_tiles = {}
                pt_tiles = {}
                sc_done = {}
                exp_end = {}
                pv_times = []       # completion time of i-th PV batch
                trans_queue = []

                def strips_gate(uidx, kt):
                    qc, h = UNITS[uidx]
                    g = strip_ready[('k', h, kt // 4)]
                    g = max(g, strip_ready[('q', h, 2 * qc)],
                            strip_ready[('q', h, 2 * qc + 1)])
                    return g + SEM

                def emit_scores(slot):
                    uidx, kt = divmod(slot, KT)
                    qc, h = UNITS[uidx]
                    gate = strips_gate(uidx, kt)
                    if slot - 2 >= 0:
                        gate = max(gate, exp_end[slot - 2] + SEM)
                    sc = pp.tile([128, QW], F32, tag="sc", bufs=2, name="sc")
                    scores(h, qc, kt, sc)
                    sc_tiles[slot] = sc
                    sc_done[slot] = pe_op(C_SCORES, gate)

                def emit_exp(slot):
                    gate = sc_done[slot] + SEM
                    if slot >= 36:
                        gate = max(gate, pv_times[slot - 36] + SEM)
                    pt = pw.tile([128, QW], F16, tag="pt", bufs=36, name="pt")
                    nc.scalar.activation(pt[:, :], sc_tiles.pop(slot)[:, :],
                                         EXP)
                    exp_end[slot] = act_op(C_EXP, gate)
                    DEBUG_EXP.append((slot, exp_end[slot] - C_EXP,
                                      sc_done[slot]))
                    pt_tiles[slot] = pt

                def pv_gate(slot):
                    uidx, kt = divmod(slot, KT)
                    g = max(exp_end[slot], strip_ready[('v', kt)])
                    if kt == 0:
                        g = max(g, memset_done[uidx])
                    return g + SEM

                def emit_pv(slot):
                    uidx, kt = divmod(slot, KT)
                    qc, h = UNITS[uidx]
                    g = pv_gate(slot)
                    pv_mms(h, kt, pt_tiles.pop(slot), last=(kt == KT - 1))
                    t_pv = pe_op(C_PV, g)
                    DEBUG_PV.append((slot, g, vt["pe"], sched["slot"]))
                    pv_times.append(t_pv)
                    if kt == KT - 1 and uidx < len(UNITS) - 1:
                        tail_dve(uidx, t_pv)

                # =========== filler inventory ===============================
                def mk_proj(st):
                    qc, qt = st // NQT, st % NQT
                    box = {}

                    def gate():
                        if (0, qc, qt) not in attnT_ready or \
                           (1, qc, qt) not in attnT_ready:
                            return None
                        return max(attnT_ready[(0, qc, qt)],
                                   attnT_ready[(1, qc, qt)]) + SEM

                    def f0():
                        box['y'] = pw.tile([128, D], F16, tag="y", bufs=4,
                                           name="y_sb")
                        proj_half(st, 0, box['y'])

                    def f1():
                        proj_half(st, 1, box['y'])
                    return {"kind": "proj", "key": ("p", st),
                            "parts": [f0, f1], "cost": C_PROJ, "next": 0,
                            "gate": gate}

                def mk_proj0(st):
                    # kf0-only partial proj of a qc1 row-block into y2,
                    # runs mid-stream once attnT[0] qc1 is transposed
                    qt = st - NQT
                    box = {}

                    def gate():
                        if (0, 1, qt) not in attnT_ready:
                            return None
                        return max(attnT_ready[(0, 1, qt)],
                                   dma_t["wp0"]) + SEM

                    def part(nn):
                        def f():
                            if nn == 0:
                                box['y'] = pw.tile([128, D], F16, tag="y",
                                                   bufs=4, name="y2_sb")
                            psy = pp.tile([128, 512], F32, tag="ch", bufs=2,
                                          name="psy0")
                            nc.tensor.matmul(
                                psy[:, :],
                                attnT[0][:, st * 128:(st + 1) * 128],
                                wp_sb[:, nn * 512:nn * 512 + 512],
                                start=True, stop=True)
                            t_pe = pe_op(C_PROJ / 2, gate() + SEM)
                            nc.vector.tensor_copy(
                                box['y'][:, nn * 512:(nn + 1) * 512],
                                psy[:, :])
                            dve_op(C_EVAC, t_pe + SEM)
                            if nn == 1:
                                nc.sync.dma_start(
                                    out=y2[qt * 128:(qt + 1) * 128, :],
                                    in_=box['y'][:, :])
                                dma_op(2048)
                        return f
                    return {"kind": "proj", "key": ("p0", st),
                            "parts": [part(0), part(1)], "cost": C_PROJ / 2,
                            "next": 0, "gate": gate}

                def unit_chain(hp, is_q, st):
                    parts = qk_chain_parts(hp, is_q, st)
                    return {"kind": "chp",
                            "key": ('q' if is_q else 'k', hp, st),
                            "parts": [p[1] for p in parts],
                            "cost": C_CHAIN4 / 2, "next": 0,
                            "gate": parts[0][2]}

                def unit_v(st):
                    parts = v_chain_parts(st)
                    return {"kind": "vp", "key": ("v", st),
                            "parts": [p[1] for p in parts],
                            "cost": C_V5, "next": 0, "gate": parts[0][2]}

                # =========== startup ========================================
                # first unit's chains, part-interleaved to fill the DMA
                # shadow; h-odd evacs ride the idle ScalarE so the DVE path
                # to the first scores is 3 casts long
                su = [qk_chain_parts(0, True, 0),
                      qk_chain_parts(0, False, 0, act_evac="all"),
                      qk_chain_parts(0, True, 1, scps=True)]
                for ci, p in ((0, 0), (0, 1), (1, 0), (1, 1), (0, 2), (0, 3),
                              (1, 2), (1, 3), (2, 0), (2, 1), (2, 2), (2, 3)):
                    su[ci][p][1]()

                fillers = []
                fillers.append(unit_chain(0, False, 1))   # k hp0 st1 (slot 4)
                fillers.append(unit_chain(0, False, 2))   # k hp0 st2 (slot 8)
                fillers.append(unit_chain(0, False, 3))   # k hp0 st3 (slot 12)
                for st in (0, 1, 2, 3):
                    fillers.append(unit_v(st))
                fillers.append(unit_chain(1, False, 0))   # k hp1 st0 (unit 2)
                fillers.append(unit_chain(1, True, 0))
                fillers.append(unit_chain(1, True, 1))
                fillers.append(unit_chain(0, True, 2))    # q hp0 st2 (unit 4)
                fillers.append(unit_chain(0, True, 3))
                for st in (4, 5, 6, 7):
                    fillers.append(unit_v(st))
                fillers.append(unit_chain(1, False, 1))
                fillers.append(unit_v(8))
                fillers.append(unit_v(9))
                fillers.append(unit_chain(1, False, 2))
                fillers.append(unit_v(10))
                fillers.append(unit_v(11))
                fillers.append(unit_chain(1, False, 3))
                for st in (12, 13, 14, 15):
                    fillers.append(unit_v(st))
                fillers.append(unit_chain(1, True, 2))    # q hp1 st2 (unit 6)
                fillers.append(unit_chain(1, True, 3))
                for st in range(8):                       # proj qc0
                    fillers.append(mk_proj(st))
                for st in range(8, 16):                   # kf0 partials qc1
                    fillers.append(mk_proj0(st))
                by_key = {u["key"]: u for u in fillers}

                pv_queue = []

                def unit_done(u):
                    return u["next"] >= len(u["parts"])

                def emit_part(u):
                    u["parts"][u["next"]]()
                    u["next"] += 1

                def force_unit(key):
                    u = by_key[key]
                    while not unit_done(u):
                        emit_part(u)

                def holds_ch(u):
                    # unit has an open chain-pool PSUM tile
                    return u["kind"] in ("chp", "vp") and not unit_done(u) \
                        and u["next"] > 0

                def next_unit():
                    """first filler whose gate is satisfied; holds the
                    chain-PSUM pool to <=2 concurrently open units."""
                    while fillers and unit_done(fillers[0]):
                        fillers.pop(0)
                    n_open = sum(1 for u in fillers if holds_ch(u))
                    for u in fillers:
                        if unit_done(u):
                            continue
                        if u["kind"] in ("chp", "vp") and u["next"] == 0 \
                           and n_open >= 2:
                            continue
                        if u["gate"]() is not None:
                            return u
                    return None

                def ensure_strips(slot):
                    uidx, kt = divmod(slot, KT)
                    qc, h = UNITS[uidx]
                    hp = h // 2
                    for key in (('k', hp, kt // 4), ('q', hp, 2 * qc),
                                ('q', hp, 2 * qc + 1)):
                        if (key[0], h, key[2]) not in strip_ready:
                            if key in by_key:
                                force_unit(key)

                def force_pv():
                    slot = pv_queue[0]
                    uidx, kt = divmod(slot, KT)
                    if ('v', kt) not in strip_ready:
                        force_unit(("v", kt))
                    emit_pv(pv_queue.pop(0))

                def try_work(deadline):
                    # 1. pending transposes (cheap, unlock proj fillers)
                    if trans_queue:
                        hp, qc, qt, u = trans_queue[0]
                        g = norm_done[u] + SEM
                        if (sched["drain"] or g <= vt["pe"] + 100) and \
                           max(vt["pe"], g) + C_TRANS <= deadline:
                            trans_queue.pop(0)
                            emit_transpose(hp, qc, qt, u)
                            return True
                    # 2. PV vs filler by pt-backlog pressure
                    target = 8 if sched["slot"] < 100 else 1
                    order = ("pv", "fill") if len(pv_queue) > target \
                        else ("fill", "pv")
                    for what in order:
                        if what == "pv" and pv_queue:
                            slot = pv_queue[0]
                            uidx, kt = divmod(slot, KT)
                            if ('v', kt) in strip_ready:
                                g = pv_gate(slot)
                                if max(vt["pe"], g) + C_PV <= deadline:
                                    emit_pv(pv_queue.pop(0))
                                    return True
                        elif what == "fill":
                            u = next_unit()
                            if u is not None:
                                g = u["gate"]()
                                if g is not None and \
                                   max(vt["pe"], g) + u["cost"] <= deadline:
                                    emit_part(u)
                                    return True
                    return False

                # slot 0 split into 512-wide halves: half A is emitted
                # before the q-st1 chain so the exp stream starts on the
                # first two chains alone
                sc0 = pp.tile([128, QW], F32, tag="sc", bufs=2, name="sc")
                pt0 = pw.tile([128, QW], F16, tag="pt", bufs=36, name="pt")
                t_half = {}

                def emit_half(nn):
                    gate = max(strip_ready[('k', 0, 0)],
                               strip_ready[('q', 0, nn)]) + SEM
                    nc.tensor.matmul(
                        sc0[:, nn * 512:(nn + 1) * 512],
                        kTt[0][0:64, :, 0:128],
                        qT[0][0:64, :, nn * 512:(nn + 1) * 512],
                        start=True, stop=True, perf_mode=DR)
                    t_pe = pe_op(C_SCORES / 2, gate)
                    nc.scalar.activation(pt0[:, nn * 512:(nn + 1) * 512],
                                         sc0[:, nn * 512:(nn + 1) * 512],
                                         EXP)
                    t_half[nn] = act_op(C_EXP / 2 + 185, t_pe + SEM)

                emit_half(0)
                emit_half(1)
                sc_tiles[0] = sc0
                sc_done[0] = vt["pe"]
                exp_end[0] = t_half[1]
                pt_tiles[0] = pt0
                DEBUG_EXP.append((0, exp_end[0], sc_done[0]))

                # =========== main slot loop =================================
                for slot in range(n_slots):
                    sched["slot"] = slot
                    if slot + 1 < n_slots:
                        ensure_strips(slot + 1)
                        emit_scores(slot + 1)
                    while pv_queue and len(pv_times) <= slot - 36:
                        force_pv()
                    if slot > 0:
                        emit_exp(slot)
                    pv_queue.append(slot)
                    if slot + 5 < n_slots:
                        ensure_strips(slot + 5)
                    deadline = vt["act"] + C_EXP - C_SCORES - WARG
                    while try_work(deadline):
                        pass

                # =========== drain ==========================================
                sched["drain"] = True
                # finish all PV (completes the last unit's accumulators)
                while pv_queue:
                    force_pv()
                t_last_pv = pv_times[-1]
                # flush leftover fillers/transposes (should be ~empty)
                DEBUG_INFO["leftover_fillers"] = sum(
                    len(u["parts"]) - u["next"] for u in fillers
                    if not unit_done(u))
                DEBUG_INFO["leftover_trans"] = len(trans_queue)
                guard = 0
                while any(not unit_done(u) for u in fillers) or trans_queue:
                    if not try_work(float("inf")):
                        guard += 1
                        if guard > 3:
                            raise RuntimeError(
                                f"drain deadlock fillers={len(fillers)} "
                                f"tq={len(trans_queue)}")
                    else:
                        guard = 0

                # last unit (qc1, h3): stage-major drain.
                # recip -> 4 norms (DVE) -> 8 transposes (PE) -> 8 copies
                # (Act) -> kf1-projs (even sts: sc-pool [128,1024]; odd sts:
                # ch-pool halves) -> evacs balanced across DVE/Act -> y DMA.
                nc.vector.reciprocal(rec_sb[:, 1, :], den[:, :, 1])
                dve_op(C_RECIP, t_last_pv + SEM)
                rec_bc = rec_sb[:, 1, :].rearrange(
                    "p (q o) -> p q o", o=1).broadcast_to((128, NQT, HD))
                acc_v = acc.rearrange("p (q c) -> p q c", c=HD)
                t_norm = {}
                for w in range(4):
                    nc.vector.tensor_tensor(
                        out=attn_sb[1][:, 2 * w:2 * w + 2, HD:2 * HD],
                        in0=acc_v[:, 2 * w:2 * w + 2, :],
                        in1=rec_bc[:, 2 * w:2 * w + 2, :], op=MUL)
                    t_norm[w] = dve_op(C_NORM / 4 + 60)
                trs = {}
                t_tr = {}
                for qt in range(NQT):
                    tr = pp.tile([128, 128], F16, tag="ch", bufs=2,
                                 name="tr")
                    nc.tensor.transpose(tr[:, :], attn_sb[1][:, qt, :],
                                        eye_sb[:, :])
                    trs[qt] = tr
                    t_tr[qt] = pe_op(C_TRANS, t_norm[qt // 2] + SEM)
                t_cp = {}
                for qt in range(NQT):
                    dst = attnT[1][:, QW + qt * 128:QW + (qt + 1) * 128]
                    nc.scalar.copy(dst, trs.pop(qt)[:, :])
                    t_cp[qt] = act_op(300, t_tr[qt] + SEM)
                for qt in range(NQT):
                    st = NQT + qt
                    y_sb = pw.tile([128, D], F16, tag="y", bufs=4,
                                   name="y_sb")
                    if qt % 2 == 0:
                        psy = pp.tile([128, QW], F32, tag="sc", bufs=2,
                                      name="psyd")
                        for nn in range(2):
                            nc.tensor.matmul(
                                psy[:, nn * 512:(nn + 1) * 512],
                                attnT[1][:, st * 128:(st + 1) * 128],
                                wp_sb[:, D + nn * 512:D + nn * 512 + 512],
                                start=True, stop=True)
                        t_pe = pe_op(C_PROJ, t_cp[qt] + SEM)
                        if qt % 4 == 0:
                            nc.vector.tensor_copy(y_sb[:, :], psy[:, :])
                            dve_op(1195, t_pe + SEM)
                        else:
                            nc.scalar.copy(y_sb[:, :], psy[:, :])
                            act_op(1040, t_pe + SEM)
                    else:
                        # borrow the dead acc/den banks for extra depth
                        tags = ("acc", "den") if qt % 4 == 1 else ("ch", "ch")
                        for nn in range(2):
                            psy = pp.tile([128, 512], F32, tag=tags[nn],
                                          bufs=1 if tags[nn] != "ch" else 2,
                                          name="psydh")
                            nc.tensor.matmul(
                                psy[:, :],
                                attnT[1][:, st * 128:(st + 1) * 128],
                                wp_sb[:, D + nn * 512:D + nn * 512 + 512],
                                start=True, stop=True)
                            t_pe = pe_op(C_PROJ / 2, t_cp[qt] + SEM)
                            if nn == 0:
                                nc.vector.tensor_copy(
                                    y_sb[:, nn * 512:(nn + 1) * 512],
                                    psy[:, :])
                                dve_op(C_EVAC, t_pe + SEM)
                            else:
                                nc.scalar.copy(
                                    y_sb[:, nn * 512:(nn + 1) * 512],
                                    psy[:, :])
                                act_op(612, t_pe + SEM)
                    nc.sync.dma_start(out=y[st * 128:(st + 1) * 128, :],
                                      in_=y_sb[:, :])
                    dma_op(2048)
    nc.compile()
    return nc


_NC_CACHE = None


def _get_nc():
    global _NC_CACHE
    if _NC_CACHE is None:
        _NC_CACHE = _build()
    return _NC_CACHE


def kernel(x, w_qkv, b_qkv, w_proj, b_proj):
    x = np.ascontiguousarray(np.asarray(x, dtype=np.float32))
    w_qkv = np.asarray(w_qkv, dtype=np.float32)
    b_qkv = np.asarray(b_qkv, dtype=np.float32)
    w_proj = np.asarray(w_proj, dtype=np.float32)
    b_proj = np.asarray(b_proj, dtype=np.float32)

    onec_np = np.ones((128, 1), np.float16)
    eye_np = np.eye(128, dtype=np.float16)

    in_maps = []
    for c in range(N_CORES):
        b = c // 4
        g = c % 4
        heads = [4 * g + i for i in range(TP)]
        # w_qkv cols: head h -> q [h*192, +64), k [+64, +128), v [+128, +192)
        qcols = np.concatenate([np.arange(h * 192, h * 192 + 64)
                                for h in heads])
        kcols = qcols + 64
        vcols = qcols + 128
        # wqk col layout: [q-hp0 | k-hp0 | q-hp1 | k-hp1], pre-scaled by RS
        wqk_c = np.ascontiguousarray(np.concatenate(
            [w_qkv[:, qcols[0:128]], w_qkv[:, kcols[0:128]],
             w_qkv[:, qcols[128:256]], w_qkv[:, kcols[128:256]]],
            axis=1) * RS).astype(np.float16)
        wv_c = np.ascontiguousarray(w_qkv[:, vcols]).astype(np.float16)
        bq = (b_qkv[qcols] * RS).reshape(2, 128)
        bk = (b_qkv[kcols] * RS).reshape(2, 128)
        bqk_c = np.ascontiguousarray(
            np.stack([bq[0], bq[1], bk[0], bk[1]], axis=1)).astype(np.float32)
        # v bias rides a matmul against the junk 0.125-row: pre-multiply by 8
        bv_c = np.ascontiguousarray(
            b_qkv[vcols].reshape(1, 256) * 8.0).astype(np.float16)
        prow = np.concatenate([np.arange(h * 64, h * 64 + 64) for h in heads])
        wp_c = np.ascontiguousarray(w_proj[prow, :]).astype(np.float16)
        xT_c = np.ascontiguousarray(x[b].T).astype(np.float16)
        xT8_c = np.ascontiguousarray(x[b].T).astype('float8_e4m3fn')
        in_maps.append({
            "xT": xT_c, "xT8": xT8_c, "wqk": wqk_c, "wv": wv_c, "wp": wp_c,
            "bqk": bqk_c, "bv": bv_c,
            "onec_in": onec_np, "eye_in": eye_np,
        })

    nc = _get_nc()
    res = run_bass_kernel_spmd(nc, in_maps, list(range(N_CORES)))
    out = np.zeros((B, S, D), dtype=np.float32)
    for c in range(N_CORES):
        out[c // 4] += res.results[c]["y"].astype(np.float32)
        out[c // 4, QW:] += res.results[c]["y2"].astype(np.float32)
    out += b_proj
    return out


# revision 53
# speedup vs baseline: 1.0060x; 1.0060x over previous
"""Trainium2 Bass kernel for AltAttention (B=2, S=2048, D=1024, 16 heads).

Distribution over 8 NeuronCores: data-parallel over batch (2) x
tensor-parallel over heads (4 heads/core).

Per-core pipeline (cost-model-aware design):
  - QKV projection chains (PSUM accumulation over D). The hp0 (heads
    0-1) q/k chains read an fp8 copy of x^T that is DMA'd first (half
    the bytes of fp16), so the first scores tile is ready ~8.5us in;
    v chains and hp1 chains read the fp16 x^T for accuracy.
  - q/k evacuated to fp8e4m3 in a [64, 2, S] layout (hd split over the
    two DoubleRow k-subtiles; subtile 1 zero-padded) with bias and
    sqrt(1/32)-scale folded in, so the scores matmul runs in fp8
    DoubleRow mode at 0.5 cycles/row. The h-even strip evacuation is
    emitted before the h-odd one so the first unit's scores gate on a
    single evac chain.
  - Scores land as [128 k, 1024 q] PSUM tiles; the ScalarE exp stream
    (128 x [128,1024] activations, ~133 us) is the critical engine; a
    build-time virtual-clock scheduler (with a 2-stage HWDGE+transfer
    DMA queue model) paces every other engine's emission so the exp
    stream never starves.
  - PV is *flipped*: pt slices [k,q] are the stationary operand, v
    strips [k,64] the moving one, so each accumulation step streams
    only 64+1 rows. Accumulators for 8 q-tiles share one PSUM bank via
    memset + start=False accumulation; a parallel 1-column matmul
    accumulates the softmax denominators.
  - Normalize (DVE broadcast multiply by 1/denom), PE-transpose to
    [hd,q], output projection, fp16 partial-y DMA; host sums 4
    partials per batch and adds b_proj. The last unit's tail runs as
    4 per-qt-pair waves with evacuation work round-robined between
    ScalarE (idle after the last exp) and DVE.
"""
import numpy as np

import concourse.bacc as bacc
import concourse.mybir as mybir
from concourse.tile import TileContext
from concourse.bass_utils import run_bass_kernel_spmd

B = 2
S = 2048
D = 1024
H = 16
HD = 64
SCALE = D ** (-0.5)
RS = SCALE ** 0.5          # sqrt-scale folded into both q and k
N_CORES = 8
TP = 4                     # heads per core
F32 = mybir.dt.float32
F16 = mybir.dt.float16
F8 = mybir.dt.float8e4
EXP = mybir.ActivationFunctionType.Exp
DR = mybir.MatmulPerfMode.DoubleRow
MUL = mybir.AluOpType.mult
ADD = mybir.AluOpType.add

KO = D // 128              # 8 contraction tiles over D
KT = S // 128              # 16 key tiles
QC = 2                     # q chunks of 1024
QW = S // QC               # 1024
NQT = QW // 128            # 8 q-subtiles per chunk

# virtual-clock cost constants (ns, slightly pessimistic for PE)
C_SCORES = 222             # 2 DoubleRow mms [512] @ 106.7 + decode
C_PV = 250                 # 8x(64-mm + 1-mm) + decode
C_CHAIN4 = 880             # 4 x 512-mm
C_V4 = 440                 # 4 x 256-mm
C_V5 = 550                 # 4 x 256-mm + bias mm
C_PROJ = 440               # 2 x 512-mm
C_TRANS = 56               # 128-row fp16 transpose
C_EXP = 1041               # [128,1024] exp from PSUM
C_EVAC = 660               # DVE 512-free PSUM-read op
C_VEVAC = 394              # DVE 256-free PSUM-read op
C_RECIP = 135
C_NORM = 660
C_MEMA = 660
C_MEMD = 135
C_TCOPY = 260              # DVE [128,128] psum->sbuf copy
SEM = 250                  # cross-engine semaphore latency
WARG = 340                 # scheduler reserve margin


DEBUG_EXP = []
DEBUG_PV = []
DEBUG_INFO = {}


def _build():
    DEBUG_EXP.clear()
    DEBUG_PV.clear()
    nc = bacc.Bacc("TRN2", target_bir_lowering=False, debug=False,
                   num_devices=N_CORES)

    xT = nc.dram_tensor("xT", [D, S], F16, kind="ExternalInput")
    xT8 = nc.dram_tensor("xT8", [D, S], F8, kind="ExternalInput")
    # per-core weight slices, host-prearranged:
    #   wqk: [D, 512] cols = [q-hp0 | k-hp0 | q-hp1 | k-hp1] (128 each)
    #   wv:  [D, 256] cols = v h0..h3
    wqk = nc.dram_tensor("wqk", [D, 512], F16, kind="ExternalInput")
    wv = nc.dram_tensor("wv", [D, 256], F16, kind="ExternalInput")
    wp = nc.dram_tensor("wp", [TP * HD, D], F16, kind="ExternalInput")
    # bqk[:, c]: per-partition bias (pre-scaled by RS): cols q-hp0,q-hp1,k-hp0,k-hp1
    bqk = nc.dram_tensor("bqk", [128, 4], F32, kind="ExternalInput")
    bv = nc.dram_tensor("bv", [1, 256], F16, kind="ExternalInput")
    onec_in = nc.dram_tensor("onec_in", [128, 1], F16, kind="ExternalInput")
    eye_in = nc.dram_tensor("eye_in", [128, 128], F16, kind="ExternalInput")
    y = nc.dram_tensor("y", [S, D], F16, kind="ExternalOutput")
    # kf0 partial of the qc1 rows, computed mid-stream; host adds to y rows
    y2 = nc.dram_tensor("y2", [QW, D], F16, kind="ExternalOutput")

    with TileContext(nc) as tc, \
         nc.allow_low_precision(reason="fp16/fp8 PE operands; fp16 partial y"):
        with tc.tile_pool(name="pconst", bufs=1) as pc, \
             tc.tile_pool(name="pmain", bufs=1) as pm, \
             tc.tile_pool(name="pp", bufs=1, space="PSUM") as pp:
            # ---- resident constants / weights ----
            wqk_sb = pc.tile([128, KO, 512], F16, name="wqk_sb")
            wv_sb = pc.tile([128, KO, 256], F16, name="wv_sb")
            wp_sb = pc.tile([128, 2 * D], F16, name="wp_sb")
            bqk_sb = pc.tile([128, 4], F32, name="bqk_sb")
            bv_sb = pc.tile([1, 256], F16, name="bv_sb")
            onec_sb = pc.tile([128, 1], F16, name="onec_sb")
            eye_sb = pc.tile([128, 128], F16, name="eye_sb")
            junk_sb = pc.tile([128, 512], F16, name="junk_sb")
            junk2_sb = pc.tile([128, 128], F16, name="junk2_sb")

            # ---- persistent activations ----
            xT_sb = pm.tile([128, KO, S], F16, name="xT_sb")
            xT8_sb = pm.tile([128, KO, S], F8, name="xT8_sb")
            qT = [pm.tile([64, 2, S], F8, name=f"qT{h}") for h in range(TP)]
            kTt = [pm.tile([64, 2, S], F8, name=f"kT{h}") for h in range(TP)]
            v_view = pm.tile([128, KT, TP, HD], F16, name="v_aug")
            attn_sb = [pm.tile([128, NQT, 128], F16, name=f"attn{hp}")
                       for hp in range(2)]
            attnT = [pm.tile([128, S], F16, name=f"attnT{hp}")
                     for hp in range(2)]
            rec_sb = pm.tile([128, 2, NQT], F32, name="rec_sb")

            with tc.tile_pool(name="pwork", bufs=1) as pw:
                # ---- virtual clocks ----
                vt = {"pe": 0.0, "act": 0.0, "dve": 0.0}
                dmaq = {"hw": 700.0, "t": 0.0}

                def pe_op(cost, gate=0.0):
                    vt["pe"] = max(vt["pe"], gate) + cost
                    return vt["pe"]

                def dve_op(cost, gate=0.0):
                    vt["dve"] = max(vt["dve"], gate) + cost
                    return vt["dve"]

                def act_op(cost, gate=0.0):
                    vt["act"] = max(vt["act"], gate) + cost
                    return vt["act"]

                def dma_op(pp_bytes, elem_bytes=512):
                    # 2-stage queue: serial HWDGE descriptor gen (625ns),
                    # serial DMA transfer at ~0.3555 ns/B-per-partition
                    # (2x for <512B contiguous runs), then +941ns before
                    # the completion semaphore is visible to consumers.
                    dmaq["hw"] += 625.0
                    start = max(dmaq["t"], dmaq["hw"] + 650.0)
                    mult = 2.0 if elem_bytes < 512 else 1.0
                    dmaq["t"] = start + max(pp_bytes * 0.3555 * mult, 56.0)
                    return dmaq["t"] + 941.0

                acc = pp.tile([128, 512], F32, tag="acc", bufs=1, name="acc")
                den = pp.tile([128, NQT, 2], F32, tag="den", bufs=1,
                              name="den")

                # =========== warmup: PE p-state + ACT exp table =============
                nc.gpsimd.memset(junk_sb[:, :], 0.125)
                wups = [pp.tile([128, 512], F32, tag="ch", bufs=2,
                                name="wup") for _ in range(2)]
                for i in range(5):
                    nc.tensor.matmul(wups[i % 2][:, :], junk_sb[:, 0:128],
                                     junk_sb[:, :], start=True, stop=True)
                nc.scalar.activation(junk2_sb[:, :], junk_sb[:, 0:128], EXP)

                # =========== input DMAs =====================================
                # ordered so the first unit's (qc0, h0) dependencies land
                # first: w-q-hp0, bias, x8 st0, w-k-hp0, x8 st1, ...
                wqkr = wqk.rearrange("(k p) c -> p k c", p=128)
                xTr = xT.rearrange("(k p) s -> p k s", p=128)
                xT8r = xT8.rearrange("(k p) s -> p k s", p=128)
                dma_t = {}
                x8_t = {}
                # 256-col weight slices keep DRAM rows 512B-contiguous
                nc.sync.dma_start(out=wqk_sb[:, 0:4, 0:256],
                                  in_=wqkr[:, 0:4, 0:256])
                dma_t["w0a"] = dma_op(2048)
                nc.sync.dma_start(
                    out=xT8_sb[:, 0:4, 0:512], in_=xT8r[:, 0:4, 0:512])
                x8_t[(0, "a")] = dma_op(2048)
                nc.sync.dma_start(
                    out=xT8_sb[:, 4:8, 0:512], in_=xT8r[:, 4:8, 0:512])
                x8_t[(0, "b")] = x8_t[0] = dma_op(2048)
                nc.sync.dma_start(out=wqk_sb[:, 4:8, 0:256],
                                  in_=wqkr[:, 4:8, 0:256])
                dma_t["w0b"] = dma_op(2048)
                nc.sync.dma_start(out=bqk_sb[:], in_=bqk[:, :])
                dma_op(16, 16)
                for half in (0, 1):
                    nc.sync.dma_start(
                        out=xT8_sb[:, 4 * half:4 * half + 4, 512:1024],
                        in_=xT8r[:, 4 * half:4 * half + 4, 512:1024])
                    x8_t[(1, "ab"[half])] = dma_op(2048)
                x8_t[1] = x8_t[(1, "b")]
                for st in (2, 3):
                    nc.sync.dma_start(
                        out=xT8_sb[:, :, st * 512:(st + 1) * 512],
                        in_=xT8r[:, :, st * 512:(st + 1) * 512])
                    x8_t[st] = dma_op(4096)
                nc.sync.dma_start(out=bv_sb[:], in_=bv[:, :])
                dma_op(512)
                nc.sync.dma_start(out=onec_sb[:], in_=onec_in[:, :])
                dma_op(2, 2)
                nc.sync.dma_start(out=wv_sb[:, :, :],
                                  in_=wv.rearrange("(k p) c -> p k c", p=128))
                dma_t["wv"] = dma_op(4096)
                nc.sync.dma_start(
                    out=xT_sb[:, :, 0:512], in_=xTr[:, :, 0:512])
                dma_t["xc0"] = dma_op(8192)
                nc.sync.dma_start(out=wqk_sb[:, :, 256:512],
                                  in_=wqkr[:, :, 256:512])
                dma_t["w1"] = dma_op(4096)
                nc.sync.dma_start(out=eye_sb[:], in_=eye_in[:, :])
                dma_op(256, 256)
                for ch in (1, 2, 3):
                    nc.sync.dma_start(
                        out=xT_sb[:, :, ch * 512:(ch + 1) * 512],
                        in_=xTr[:, :, ch * 512:(ch + 1) * 512])
                    dma_t[f"xc{ch}"] = dma_op(8192)
                for kf in range(2):
                    nc.sync.dma_start(out=wp_sb[:, kf * D:(kf + 1) * D],
                                      in_=wp[kf * 128:(kf + 1) * 128, :])
                    dma_t[f"wp{kf}"] = dma_op(2048)

                # zero the unused DoubleRow k-subtile once (SBUF, Pool ok)
                # h0 strips first: the first unit's scores read them
                for h in range(TP):
                    nc.gpsimd.memset(qT[h][0:64, 1, :], 0.0)
                    nc.gpsimd.memset(kTt[h][0:64, 1, :], 0.0)
                # zero accumulators before first use (HW PSUM is garbage)
                nc.vector.memset(acc[:, :], 0.0)
                nc.vector.memset(den[:, :, :], 0.0)
                memset_done = {0: 900.0}

                # =========== building blocks ================================
                # wqk_sb col layout: hp*256 + (0 if q else 128)
                strip_ready = {}   # ('q'|'k', h, st) / ('v', st) -> dve time
                norm_done = {}
                attnT_ready = {}   # (hp, qc, qt) -> dve time of copy
                sched = {"drain": False, "slot": 0}

                def chain_tile():
                    return pp.tile([128, 512], F32, tag="ch", bufs=2,
                                   name="chps")

                def qk_chain_parts(hp, is_q, st, scps=False,
                                   act_evac=False):
                    """parts 0-3: 2 ko-steps each; part 3 evacuates the
                    strips (h-even: DVE cast with per-partition bias;
                    weights are RS-prescaled). act_evac (startup only):
                    the h-odd bias is added in-PSUM by a masked bias-row
                    matmul so the h-odd evac is a pure ScalarE copy.
                    scps borrows an sc-pool tile."""
                    col = hp * 256 + (0 if is_q else 128)
                    if hp == 0:
                        src = xT8_sb
                        wa, wb = dma_t["w0a"], dma_t["w0b"]
                        if st <= 1:
                            xa, xb = x8_t[(st, 'a')], x8_t[(st, 'b')]
                        else:
                            xa = xb = x8_t[st]
                        gates = [max(wa, xa) + SEM] * 2 + \
                            [max(wb, xb) + SEM] * 2
                    else:
                        src = xT_sb
                        wt = dma_t["w1"]
                        gates = [max(wt, dma_t[f"xc{st}"]) + SEM] * 4
                    gate = gates[0]
                    box = {}

                    def evac(sub):
                        h = 2 * hp + sub
                        blk = 0 if is_q else 1
                        dstl = qT if is_q else kTt
                        dst = dstl[h][0:64, 0, st * 512:(st + 1) * 512]
                        srcp = box['ps'][64 * sub:64 * sub + 64, 0:512]
                        if act_evac == "all" or (sub == 1 and act_evac):
                            nc.scalar.activation(
                                dst, srcp,
                                mybir.ActivationFunctionType.Identity,
                                bias=bqk_sb[64 * sub:64 * sub + 64,
                                            2 * blk + hp:2 * blk + hp + 1])
                            strip_ready[('q' if is_q else 'k', h, st)] = \
                                act_op(612, box['t_pe'] + SEM)
                            return
                        bias = bqk_sb[64 * sub:64 * sub + 64,
                                      2 * blk + hp:2 * blk + hp + 1]
                        nc.vector.tensor_scalar(
                            out=dst, in0=srcp, scalar1=bias, scalar2=None,
                            op0=ADD)
                        strip_ready[('q' if is_q else 'k', h, st)] = \
                            dve_op(C_EVAC, box['t_pe'] + SEM)

                    def part(p):
                        def f():
                            if p == 0:
                                if scps:
                                    box['ps'] = pp.tile([128, QW], F32,
                                                        tag="sc", bufs=2,
                                                        name="chsc")
                                else:
                                    box['ps'] = chain_tile()
                            ps = box['ps']
                            for ko in (2 * p, 2 * p + 1):
                                nc.tensor.matmul(
                                    ps[:, 0:512],
                                    wqk_sb[:, ko, col:col + 128],
                                    src[:, ko, st * 512:(st + 1) * 512],
                                    start=(ko == 0), stop=(ko == KO - 1))
                            box['t_pe'] = pe_op(C_CHAIN4 / 2, gates[p])
                            if p == 3:
                                evac(0)
                                evac(1)
                        return ("chp", f, lambda: gate)
                    return [part(p) for p in range(4)]

                def v_chain_parts(st):
                    gate = max(dma_t["wv"], dma_t[f"xc{st // 4}"]) + SEM
                    box = {}

                    def f0():
                        box['ps'] = chain_tile()
                        for ko in range(4):
                            nc.tensor.matmul(
                                box['ps'][:, 0:256],
                                xT_sb[:, ko, st * 128:(st + 1) * 128],
                                wv_sb[:, ko, :],
                                start=(ko == 0), stop=False)
                        pe_op(C_V4, gate)

                    def f1():
                        ps = box['ps']
                        for ko in range(4, 8):
                            nc.tensor.matmul(
                                ps[:, 0:256],
                                xT_sb[:, ko, st * 128:(st + 1) * 128],
                                wv_sb[:, ko, :],
                                start=False, stop=False)
                        nc.tensor.matmul(ps[:, 0:256], junk_sb[0:1, 0:128],
                                         bv_sb[0:1, :], start=False,
                                         stop=True)
                        t_pe = pe_op(C_V5)
                        nc.vector.tensor_copy(
                            v_view[:, st, :, :],
                            ps.rearrange("p (h c) -> p h c", c=HD)[:, 0:4, :])
                        strip_ready[('v', st)] = dve_op(C_VEVAC, t_pe + SEM)
                    return [("vp", f0, lambda: gate), ("vp", f1, lambda: gate)]

                def scores(h, qc, kt, sc):
                    for nn in range(2):
                        nc.tensor.matmul(
                            sc[:, nn * 512:(nn + 1) * 512],
                            kTt[h][0:64, :, kt * 128:(kt + 1) * 128],
                            qT[h][0:64, :,
                                  qc * QW + nn * 512:qc * QW + (nn + 1) * 512],
                            start=True, stop=True, perf_mode=DR)

                def pv_mms(h, kt, pt, last):
                    for qt in range(NQT):
                        st_ap = pt[:, qt * 128:(qt + 1) * 128]
                        nc.tensor.matmul(
                            acc[:, qt * HD:(qt + 1) * HD],
                            st_ap, v_view[:, kt, h, :],
                            start=False, stop=last, skip_group_check=True)
                        nc.tensor.matmul(
                            den[:, qt, h % 2:h % 2 + 1],
                            st_ap, onec_sb[:, 0:1],
                            start=False, stop=last, skip_group_check=True)

                def tail_dve(uidx, t_pv):
                    """end-of-unit: normalize, reset accumulators, queue
                    transposes. Used for all units except the last."""
                    qc, h = UNITS[uidx]
                    hp, sub = h // 2, h % 2
                    nc.vector.reciprocal(rec_sb[:, sub, :], den[:, :, sub])
                    dve_op(C_RECIP, t_pv + SEM)
                    rec_bc = rec_sb[:, sub, :].rearrange(
                        "p (q o) -> p q o", o=1).broadcast_to((128, NQT, HD))
                    acc_v = acc.rearrange("p (q c) -> p q c", c=HD)
                    nc.vector.tensor_tensor(
                        out=attn_sb[hp][:, :, sub * HD:(sub + 1) * HD],
                        in0=acc_v, in1=rec_bc, op=MUL)
                    norm_done[uidx] = dve_op(C_NORM)
                    nc.vector.memset(acc[:, :], 0.0)
                    dve_op(C_MEMA)
                    nc.vector.memset(den[:, :, sub], 0.0)
                    memset_done[uidx + 1] = dve_op(C_MEMD)
                    if h % 2 == 1:
                        for qt in range(NQT):
                            trans_queue.append((hp, qc, qt, uidx))

                def emit_transpose(hp, qc, qt, uidx):
                    tr = pp.tile([128, 128], F16, tag="ch", bufs=2, name="tr")
                    nc.tensor.transpose(tr[:, :], attn_sb[hp][:, qt, :],
                                        eye_sb[:, :])
                    t_pe = pe_op(C_TRANS, norm_done[uidx] + SEM)
                    dst = attnT[hp][:, qc * QW + qt * 128:
                                    qc * QW + (qt + 1) * 128]
                    nc.vector.tensor_copy(dst, tr[:, :])
                    attnT_ready[(hp, qc, qt)] = dve_op(C_TCOPY, t_pe + SEM)

                def proj_half(st, nn, y_sb):
                    qc, qt = st // NQT, st % NQT
                    gate = max(attnT_ready[(0, qc, qt)],
                               attnT_ready[(1, qc, qt)],
                               dma_t["wp1"]) + SEM
                    psy = pp.tile([128, 512], F32, tag="ch", bufs=2,
                                  name="psy")
                    for kf in range(2):
                        nc.tensor.matmul(
                            psy[:, :],
                            attnT[kf][:, st * 128:(st + 1) * 128],
                            wp_sb[:, kf * D + nn * 512:kf * D + nn * 512 + 512],
                            start=(kf == 0), stop=(kf == 1))
                    t_pe = pe_op(C_PROJ, gate)
                    nc.vector.tensor_copy(
                        y_sb[:, nn * 512:(nn + 1) * 512], psy[:, :])
                    dve_op(C_EVAC, t_pe + SEM)
                    if nn == 1:
                        nc.sync.dma_start(out=y[st * 128:(st + 1) * 128, :],
                                          in_=y_sb[:, :])
                        dma_op(2048)

                # =========== unit / slot bookkeeping ========================
                UNITS = [(qc, h) for qc in range(QC) for h in range(TP)]
                n_slots = len(UNITS) * KT
                sc_tiles = {}
                pt_tiles = {}
                sc_done = {}
                exp_end = {}
                pv_times = []       # completion time of i-th PV batch
                trans_queue = []

                def strips_gate(uidx, kt):
                    qc, h = UNITS[uidx]
                    g = strip_ready[('k', h, kt // 4)]
                    g = max(g, strip_ready[('q', h, 2 * qc)],
                            strip_ready[('q', h, 2 * qc + 1)])
                    return g + SEM

                def emit_scores(slot):
                    uidx, kt = divmod(slot, KT)
                    qc, h = UNITS[uidx]
                    gate = strips_gate(uidx, kt)
                    if slot - 2 >= 0:
                        gate = max(gate, exp_end[slot - 2] + SEM)
                    sc = pp.tile([128, QW], F32, tag="sc", bufs=2, name="sc")
                    scores(h, qc, kt, sc)
                    sc_tiles[slot] = sc
                    sc_done[slot] = pe_op(C_SCORES, gate)

                def emit_exp(slot):
                    gate = sc_done[slot] + SEM
                    if slot >= 36:
                        gate = max(gate, pv_times[slot - 36] + SEM)
                    pt = pw.tile([128, QW], F16, tag="pt", bufs=36, name="pt")
                    nc.scalar.activation(pt[:, :], sc_tiles.pop(slot)[:, :],
                                         EXP)
                    exp_end[slot] = act_op(C_EXP, gate)
                    DEBUG_EXP.append((slot, exp_end[slot] - C_EXP,
                                      sc_done[slot]))
                    pt_tiles[slot] = pt

                def pv_gate(slot):
                    uidx, kt = divmod(slot, KT)
                    g = max(exp_end[slot], strip_ready[('v', kt)])
                    if kt == 0:
                        g = max(g, memset_done[uidx])
                    return g + SEM

                def emit_pv(slot):
                    uidx, kt = divmod(slot, KT)
                    qc, h = UNITS[uidx]
                    g = pv_gate(slot)
                    pv_mms(h, kt, pt_tiles.pop(slot), last=(kt == KT - 1))
                    t_pv = pe_op(C_PV, g)
                    DEBUG_PV.append((slot, g, vt["pe"], sched["slot"]))
                    pv_times.append(t_pv)
                    if kt == KT - 1 and uidx < len(UNITS) - 1:
                        tail_dve(uidx, t_pv)

                # =========== filler inventory ===============================
                def mk_proj(st):
                    qc, qt = st // NQT, st % NQT
                    box = {}

                    def gate():
                        if (0, qc, qt) not in attnT_ready or \
                           (1, qc, qt) not in attnT_ready:
                            return None
                        return max(attnT_ready[(0, qc, qt)],
                                   attnT_ready[(1, qc, qt)]) + SEM

                    def f0():
                        box['y'] = pw.tile([128, D], F16, tag="y", bufs=4,
                                           name="y_sb")
                        proj_half(st, 0, box['y'])

                    def f1():
                        proj_half(st, 1, box['y'])
                    return {"kind": "proj", "key": ("p", st),
                            "parts": [f0, f1], "cost": C_PROJ, "next": 0,
                            "gate": gate}

                def mk_proj0(st):
                    # kf0-only partial proj of a qc1 row-block into y2,
                    # runs mid-stream once attnT[0] qc1 is transposed
                    qt = st - NQT
                    box = {}

                    def gate():
                        if (0, 1, qt) not in attnT_ready:
                            return None
                        return max(attnT_ready[(0, 1, qt)],
                                   dma_t["wp0"]) + SEM

                    def part(nn):
                        def f():
                            if nn == 0:
                                box['y'] = pw.tile([128, D], F16, tag="y",
                                                   bufs=4, name="y2_sb")
                            psy = pp.tile([128, 512], F32, tag="ch", bufs=2,
                                          name="psy0")
                            nc.tensor.matmul(
                                psy[:, :],
                                attnT[0][:, st * 128:(st + 1) * 128],
                                wp_sb[:, nn * 512:nn * 512 + 512],
                                start=True, stop=True)
                            t_pe = pe_op(C_PROJ / 2, gate() + SEM)
                            nc.vector.tensor_copy(
                                box['y'][:, nn * 512:(nn + 1) * 512],
                                psy[:, :])
                            dve_op(C_EVAC, t_pe + SEM)
                            if nn == 1:
                                nc.sync.dma_start(
                                    out=y2[qt * 128:(qt + 1) * 128, :],
                                    in_=box['y'][:, :])
                                dma_op(2048)
                        return f
                    return {"kind": "proj", "key": ("p0", st),
                            "parts": [part(0), part(1)], "cost": C_PROJ / 2,
                            "next": 0, "gate": gate}

                def unit_chain(hp, is_q, st):
                    parts = qk_chain_parts(hp, is_q, st)
                    return {"kind": "chp",
                            "key": ('q' if is_q else 'k', hp, st),
                            "parts": [p[1] for p in parts],
                            "cost": C_CHAIN4 / 2, "next": 0,
                            "gate": parts[0][2]}

                def unit_v(st):
                    parts = v_chain_parts(st)
                    return {"kind": "vp", "key": ("v", st),
                            "parts": [p[1] for p in parts],
                            "cost": C_V5, "next": 0, "gate": parts[0][2]}

                # =========== startup ========================================
                # first unit's chains, part-interleaved to fill the DMA
                # shadow; h-odd evacs ride the idle ScalarE so the DVE path
                # to the first scores is 3 casts long
                su = [qk_chain_parts(0, True, 0),
                      qk_chain_parts(0, False, 0, act_evac="all"),
                      qk_chain_parts(0, True, 1, scps=True)]
                for ci, p in ((0, 0), (0, 1), (1, 0), (1, 1), (0, 2), (0, 3),
                              (1, 2), (1, 3), (2, 0), (2, 1), (2, 2), (2, 3)):
                    su[ci][p][1]()

                fillers = []
                fillers.append(unit_chain(0, False, 1))   # k hp0 st1 (slot 4)
                fillers.append(unit_chain(0, False, 2))   # k hp0 st2 (slot 8)
                fillers.append(unit_chain(0, False, 3))   # k hp0 st3 (slot 12)
                for st in (0, 1, 2, 3):
                    fillers.append(unit_v(st))
                fillers.append(unit_chain(1, False, 0))   # k hp1 st0 (unit 2)
                fillers.append(unit_chain(1, True, 0))
                fillers.append(unit_chain(1, True, 1))
                fillers.append(unit_chain(0, True, 2))    # q hp0 st2 (unit 4)
                fillers.append(unit_chain(0, True, 3))
                for st in (4, 5, 6, 7):
                    fillers.append(unit_v(st))
                fillers.append(unit_chain(1, False, 1))
                fillers.append(unit_v(8))
                fillers.append(unit_v(9))
                fillers.append(unit_chain(1, False, 2))
                fillers.append(unit_v(10))
                fillers.append(unit_v(11))
                fillers.append(unit_chain(1, False, 3))
                for st in (12, 13, 14, 15):
                    fillers.append(unit_v(st))
                fillers.append(unit_chain(1, True, 2))    # q hp1 st2 (unit 6)
                fillers.append(unit_chain(1, True, 3))
                for st in range(8):                       # proj qc0
                    fillers.append(mk_proj(st))
                for st in range(8, 16):                   # kf0 partials qc1
                    fillers.append(mk_proj0(st))
                by_key = {u["key"]: u for u in fillers}

                pv_queue = []

                def unit_done(u):
                    return u["next"] >= len(u["parts"])

                def emit_part(u):
                    u["parts"][u["next"]]()
                    u["next"] += 1

                def force_unit(key):
                    u = by_key[key]
                    while not unit_done(u):
                        emit_part(u)

                def holds_ch(u):
                    # unit has an open chain-pool PSUM tile
                    return u["kind"] in ("chp", "vp") and not unit_done(u) \
                        and u["next"] > 0

                def next_unit():
                    """first filler whose gate is satisfied; holds the
                    chain-PSUM pool to <=2 concurrently open units."""
                    while fillers and unit_done(fillers[0]):
                        fillers.pop(0)
                    n_open = sum(1 for u in fillers if holds_ch(u))
                    for u in fillers:
                        if unit_done(u):
                            continue
                        if u["kind"] in ("chp", "vp") and u["next"] == 0 \
                           and n_open >= 2:
                            continue
                        if u["gate"]() is not None:
                            return u
                    return None

                def ensure_strips(slot):
                    uidx, kt = divmod(slot, KT)
                    qc, h = UNITS[uidx]
                    hp = h // 2
                    for key in (('k', hp, kt // 4), ('q', hp, 2 * qc),
                                ('q', hp, 2 * qc + 1)):
                        if (key[0], h, key[2]) not in strip_ready:
                            if key in by_key:
                                force_unit(key)

                def force_pv():
                    slot = pv_queue[0]
                    uidx, kt = divmod(slot, KT)
                    if ('v', kt) not in strip_ready:
                        force_unit(("v", kt))
                    emit_pv(pv_queue.pop(0))

                def try_work(deadline):
                    # 1. pending transposes (cheap, unlock proj fillers)
                    if trans_queue:
                        hp, qc, qt, u = trans_queue[0]
                        g = norm_done[u] + SEM
                        if (sched["drain"] or g <= vt["pe"] + 100) and \
                           max(vt["pe"], g) + C_TRANS <= deadline:
                            trans_queue.pop(0)
                            emit_transpose(hp, qc, qt, u)
                            return True
                    # 2. PV vs filler by pt-backlog pressure
                    target = 8 if sched["slot"] < 100 else 1
                    order = ("pv", "fill") if len(pv_queue) > target \
                        else ("fill", "pv")
                    for what in order:
                        if what == "pv" and pv_queue:
                            slot = pv_queue[0]
                            uidx, kt = divmod(slot, KT)
                            if ('v', kt) in strip_ready:
                                g = pv_gate(slot)
                                if max(vt["pe"], g) + C_PV <= deadline:
                                    emit_pv(pv_queue.pop(0))
                                    return True
                        elif what == "fill":
                            u = next_unit()
                            if u is not None:
                                g = u["gate"]()
                                if g is not None and \
                                   max(vt["pe"], g) + u["cost"] <= deadline:
                                    emit_part(u)
                                    return True
                    return False

                # slot 0 split into 512-wide halves: half A is emitted
                # before the q-st1 chain so the exp stream starts on the
                # first two chains alone
                sc0 = pp.tile([128, QW], F32, tag="sc", bufs=2, name="sc")
                pt0 = pw.tile([128, QW], F16, tag="pt", bufs=36, name="pt")
                t_half = {}

                def emit_half(nn):
                    gate = max(strip_ready[('k', 0, 0)],
                               strip_ready[('q', 0, nn)]) + SEM
                    nc.tensor.matmul(
                        sc0[:, nn * 512:(nn + 1) * 512],
                        kTt[0][0:64, :, 0:128],
                        qT[0][0:64, :, nn * 512:(nn + 1) * 512],
                        start=True, stop=True, perf_mode=DR)
                    t_pe = pe_op(C_SCORES / 2, gate)
                    nc.scalar.activation(pt0[:, nn * 512:(nn + 1) * 512],
                                         sc0[:, nn * 512:(nn + 1) * 512],
                                         EXP)
                    t_half[nn] = act_op(C_EXP / 2 + 185, t_pe + SEM)

                emit_half(0)
                emit_half(1)
                sc_tiles[0] = sc0
                sc_done[0] = vt["pe"]
                exp_end[0] = t_half[1]
                pt_tiles[0] = pt0
                DEBUG_EXP.append((0, exp_end[0], sc_done[0]))

                # =========== main slot loop =================================
                for slot in range(n_slots):
                    sched["slot"] = slot
                    if slot + 1 < n_slots:
                        ensure_strips(slot + 1)
                        emit_scores(slot + 1)
                    while pv_queue and len(pv_times) <= slot - 36:
                        force_pv()
                    if slot > 0:
                        emit_exp(slot)
                    pv_queue.append(slot)
                    if slot + 5 < n_slots:
                        ensure_strips(slot + 5)
                    deadline = vt["act"] + C_EXP - C_SCORES - WARG
                    while try_work(deadline):
                        pass

                # =========== drain ==========================================
                sched["drain"] = True
                # finish all PV (completes the last unit's accumulators)
                while pv_queue:
                    force_pv()
                t_last_pv = pv_times[-1]
                # flush leftover fillers/transposes (should be ~empty)
                DEBUG_INFO["leftover_fillers"] = sum(
                    len(u["parts"]) - u["next"] for u in fillers
                    if not unit_done(u))
                DEBUG_INFO["leftover_trans"] = len(trans_queue)
                guard = 0
                while any(not unit_done(u) for u in fillers) or trans_queue:
                    if not try_work(float("inf")):
                        guard += 1
                        if guard > 3:
                            raise RuntimeError(
                                f"drain deadlock fillers={len(fillers)} "
                                f"tq={len(trans_queue)}")
                    else:
                        guard = 0

                # last unit (qc1, h3): stage-major drain.
                # recip -> 4 norms (DVE) -> 8 transposes (PE) -> 8 copies
                # (Act) -> kf1-projs (even sts: sc-pool [128,1024]; odd sts:
                # ch-pool halves) -> evacs balanced across DVE/Act -> y DMA.
                nc.vector.reciprocal(rec_sb[:, 1, :], den[:, :, 1])
                dve_op(C_RECIP, t_last_pv + SEM)
                rec_bc = rec_sb[:, 1, :].rearrange(
                    "p (q o) -> p q o", o=1).broadcast_to((128, NQT, HD))
                acc_v = acc.rearrange("p (q c) -> p q c", c=HD)
                t_norm = {}
                for w in range(4):
                    nc.vector.tensor_tensor(
                        out=attn_sb[1][:, 2 * w:2 * w + 2, HD:2 * HD],
                        in0=acc_v[:, 2 * w:2 * w + 2, :],
                        in1=rec_bc[:, 2 * w:2 * w + 2, :], op=MUL)
                    t_norm[w] = dve_op(C_NORM / 4 + 60)
                trs = {}
                t_tr = {}
                for qt in range(NQT):
                    tr = pp.tile([128, 128], F16, tag="ch", bufs=2,
                                 name="tr")
                    nc.tensor.transpose(tr[:, :], attn_sb[1][:, qt, :],
                                        eye_sb[:, :])
                    trs[qt] = tr
                    t_tr[qt] = pe_op(C_TRANS, t_norm[qt // 2] + SEM)
                t_cp = {}
                for qt in range(NQT):
                    dst = attnT[1][:, QW + qt * 128:QW + (qt + 1) * 128]
                    if qt % 4 == 3:
                        nc.vector.tensor_copy(dst, trs.pop(qt)[:, :])
                        t_cp[qt] = dve_op(C_TCOPY, t_tr[qt] + SEM)
                    else:
                        nc.scalar.copy(dst, trs.pop(qt)[:, :])
                        t_cp[qt] = act_op(300, t_tr[qt] + SEM)
                for qt in range(NQT):
                    st = NQT + qt
                    y_sb = pw.tile([128, D], F16, tag="y", bufs=4,
                                   name="y_sb")
                    if qt % 2 == 0:
                        psy = pp.tile([128, QW], F32, tag="sc", bufs=2,
                                      name="psyd")
                        for nn in range(2):
                            nc.tensor.matmul(
                                psy[:, nn * 512:(nn + 1) * 512],
                                attnT[1][:, st * 128:(st + 1) * 128],
                                wp_sb[:, D + nn * 512:D + nn * 512 + 512],
                                start=True, stop=True)
                        t_pe = pe_op(C_PROJ, t_cp[qt] + SEM)
                        if qt % 4 == 0:
                            nc.vector.tensor_copy(y_sb[:, :], psy[:, :])
                            dve_op(1195, t_pe + SEM)
                        else:
                            nc.scalar.copy(y_sb[:, :], psy[:, :])
                            act_op(1040, t_pe + SEM)
                    else:
                        # borrow the dead acc/den banks for extra depth
                        tags = ("acc", "den") if qt % 4 == 1 else ("ch", "ch")
                        for nn in range(2):
                            psy = pp.tile([128, 512], F32, tag=tags[nn],
                                          bufs=1 if tags[nn] != "ch" else 2,
                                          name="psydh")
                            nc.tensor.matmul(
                                psy[:, :],
                                attnT[1][:, st * 128:(st + 1) * 128],
                                wp_sb[:, D + nn * 512:D + nn * 512 + 512],
                                start=True, stop=True)
                            t_pe = pe_op(C_PROJ / 2, t_cp[qt] + SEM)
                            if nn == 0:
                                nc.vector.tensor_copy(
                                    y_sb[:, nn * 512:(nn + 1) * 512],
                                    psy[:, :])
                                dve_op(C_EVAC, t_pe + SEM)
                            else:
                                nc.scalar.copy(
                                    y_sb[:, nn * 512:(nn + 1) * 512],
                                    psy[:, :])
                                act_op(612, t_pe + SEM)
                    nc.sync.dma_start(out=y[st * 128:(st + 1) * 128, :],
                                      in_=y_sb[:, :])
                    dma_op(2048)
    nc.compile()
    return nc


_NC_CACHE = None


def _get_nc():
    global _NC_CACHE
    if _NC_CACHE is None:
        _NC_CACHE = _build()
    return _NC_CACHE


def kernel(x, w_qkv, b_qkv, w_proj, b_proj):
    x = np.ascontiguousarray(np.asarray(x, dtype=np.float32))
    w_qkv = np.asarray(w_qkv, dtype=np.float32)
    b_qkv = np.asarray(b_qkv, dtype=np.float32)
    w_proj = np.asarray(w_proj, dtype=np.float32)
    b_proj = np.asarray(b_proj, dtype=np.float32)

    onec_np = np.ones((128, 1), np.float16)
    eye_np = np.eye(128, dtype=np.float16)

    in_maps = []
    for c in range(N_CORES):
        b = c // 4
        g = c % 4
        heads = [4 * g + i for i in range(TP)]
        # w_qkv cols: head h -> q [h*192, +64), k [+64, +128), v [+128, +192)
        qcols = np.concatenate([np.arange(h * 192, h * 192 + 64)
                                for h in heads])
        kcols = qcols + 64
        vcols = qcols + 128
        # wqk col layout: [q-hp0 | k-hp0 | q-hp1 | k-hp1], pre-scaled by RS
        wqk_c = np.ascontiguousarray(np.concatenate(
            [w_qkv[:, qcols[0:128]], w_qkv[:, kcols[0:128]],
             w_qkv[:, qcols[128:256]], w_qkv[:, kcols[128:256]]],
            axis=1) * RS).astype(np.float16)
        wv_c = np.ascontiguousarray(w_qkv[:, vcols]).astype(np.float16)
        bq = (b_qkv[qcols] * RS).reshape(2, 128)
        bk = (b_qkv[kcols] * RS).reshape(2, 128)
        bqk_c = np.ascontiguousarray(
            np.stack([bq[0], bq[1], bk[0], bk[1]], axis=1)).astype(np.float32)
        # v bias rides a matmul against the junk 0.125-row: pre-multiply by 8
        bv_c = np.ascontiguousarray(
            b_qkv[vcols].reshape(1, 256) * 8.0).astype(np.float16)
        prow = np.concatenate([np.arange(h * 64, h * 64 + 64) for h in heads])
        wp_c = np.ascontiguousarray(w_proj[prow, :]).astype(np.float16)
        xT_c = np.ascontiguousarray(x[b].T).astype(np.float16)
        xT8_c = np.ascontiguousarray(x[b].T).astype('float8_e4m3fn')
        in_maps.append({
            "xT": xT_c, "xT8": xT8_c, "wqk": wqk_c, "wv": wv_c, "wp": wp_c,
            "bqk": bqk_c, "bv": bv_c,
            "onec_in": onec_np, "eye_in": eye_np,
        })

    nc = _get_nc()
    res = run_bass_kernel_spmd(nc, in_maps, list(range(N_CORES)))
    out = np.zeros((B, S, D), dtype=np.float32)
    for c in range(N_CORES):
        out[c // 4] += res.results[c]["y"].astype(np.float32)
        out[c // 4, QW:] += res.results[c]["y2"].astype(np.float32)
    out += b_proj
    return out
